# revision 4
# baseline (speedup 1.0000x reference)
"""Trainium2 kernel for nn_MultiHeadClassifier.

Math: out[i] = W[task_labels[i]] @ x[i] + b[task_labels[i]]
  x [262144, 1024] f32, task_labels [262144] int, W [8, 32, 1024], b [8, 32]

Strategy (8 NeuronCores, task-parallel, fp8 x, fp8 out):
  - Host sorts rows by task; core c processes (up to NCAP=32768) rows of
    task c, so W[c] is a per-core constant and there is NO routing on
    device at all — each core runs a plain GEMM. The ~few hundred rows
    that overflow a core's capacity are computed on host (numpy) and the
    result is merged back; bias is added on host.
  - x is sent as fp8 e3m4 (1 byte: 4 mantissa bits), quartering the
    dominant HBM traffic vs f32. W is scaled by 128 and sent as an
    e3m4 hi+lo pair, which cancels the W quantization error to second
    order.
  - hi and lo are M=32 stationaries that ACCUMULATE INTO THE SAME PSUM
    partitions (start on the first hi k-tile, stop on the last lo one),
    so the hi+lo merge happens inside PSUM for free. Four 512-row
    chunks col-tile one PSUM bank at positions 0/32/64/96; HW-probed:
    4-way M=32 streams at the same rate as 2-way M=64 (74.9 vs 74.6 us
    for 12 quads). Post-processing per 2048-row quad is then ONE ACT
    op: scaled cast PSUM * (1/128) -> e3m4 SBUF, and the whole output
    ships as fp8 (1 MiB/core vs 2.75 in the bf16+raw scheme). The
    per-core stream is SDMA-engine-bound (~26.5 GB/s x 16 engines),
    so those output bytes are pure win. Measured rel err 1.76e-2 vs
    the 2e-2 gate (HW-validated numpy sim of the exact op chain).
  - x streams on the SP HWDGE ring (first transfer is 0.5 MiB so data
    flows ~1 us earlier); fp8 out groups (256 KiB, 2 KiB/partition
    lines) ship on the ACT ring. The final 512 rows arrive split by
    k-tile halves so only 4 k-tiles of N=128 matmuls + one small cast
    + one 16 KiB DMA trail the last x byte.
"""

import sys

sys.path.insert(0, "/opt/trn_rl_repo")

import numpy as np
import ml_dtypes

import concourse.bass as bass
import concourse.tile as tile
from concourse import bacc, mybir
from concourse import bass_utils

B, D, C, T = 262144, 1024, 32, 8
NCORES = 8
P = 128
KO = D // P  # 8 contraction k-tiles
CH = 512  # rows per chunk (one col-tile position)
QR = 2048  # rows per quad (one x DMA; 4 chunks in 1 PSUM bank)
NQ = 16  # quads per core
NCAP = QR * NQ  # 32768 rows per core capacity
GQ = 4  # quads per output DMA group
WSCALE = 128.0  # power of two; exactly cancelled by the on-device 1/128
INV_WSCALE = 1.0 / WSCALE

F8 = ml_dtypes.float8_e3m4
F8LIM = 15.5  # max finite e3m4

# set by test harness to collect a profile; harness-invoked kernel() keeps it off
TRACE = False
LAST_RESULTS = None
LAST_IN_MAPS = None


def _build():
    f32 = mybir.dt.float32
    bf16 = mybir.dt.bfloat16
    f8 = mybir.dt.float8e3

    nc = bacc.Bacc("TRN2", debug=False, num_devices=NCORES)
    # quad 0 split 512+1536 rows: small first transfer -> first byte ~1us
    # earlier (descriptor gen + HBM latency off the critical path)
    xt0a_d = nc.dram_tensor("xt0a", [P, KO, CH], f8, kind="ExternalInput")
    xt0b_d = nc.dram_tensor("xt0b", [P, KO, 3 * CH], f8, kind="ExternalInput")
    # body quads 1..14: contiguous 2 MB, 16 KB/partition
    xt_d = nc.dram_tensor("xt", [NQ - 2, P, KO, QR], f8, kind="ExternalInput")
    # final quad: 1024 rows, then 512 rows, then 512 rows split by k-half
    xty_d = nc.dram_tensor("xty", [P, KO, 2 * CH], f8, kind="ExternalInput")
    xtz0_d = nc.dram_tensor("xtz0", [P, KO, CH], f8, kind="ExternalInput")
    xtzk_d = nc.dram_tensor("xtzk", [2, P, KO // 2, CH], f8, kind="ExternalInput")
    # wt[ki, ko, 0:32]=hi, [ki, ko, 32:64]=lo (host-transposed, scaled)
    wt_d = nc.dram_tensor("wt", [P, KO, 2 * C], f8, kind="ExternalInput")
    # all outputs fp8, already divided by 128 on device:
    #   out_d[g, 32j+c, 512q+r] -> row 2048*(4g+q) + 512j + r   (quads 0..11)
    out_d = nc.dram_tensor("out", [3, P, GQ * CH], f8, kind="ExternalOutput")
    #   outm_d[32j+c, 512q+r] -> row 2048*(12+q) + 512j + r     (quads 12..14)
    outm_d = nc.dram_tensor("outm", [P, 3 * CH], f8, kind="ExternalOutput")
    #   outy_d[32jj+c, r] -> row 30720 + 512jj + r              (jj=0,1)
    outy_d = nc.dram_tensor("outy", [2 * C, CH], f8, kind="ExternalOutput")
    #   outz0_d[32s+c, r] -> row 31744 + 128s + r               (s=0..3)
    outz0_d = nc.dram_tensor("outz0", [P, CH // 4], f8, kind="ExternalOutput")
    #   outzk_d[32s+c, r] -> row 32256 + 128s + r
    outzk_d = nc.dram_tensor("outzk", [P, CH // 4], f8, kind="ExternalOutput")

    with tile.TileContext(nc) as tc:
        with (
            tc.tile_pool(name="consts", bufs=1) as consts,
            tc.tile_pool(name="xpool", bufs=8) as xpool,
            tc.tile_pool(name="tailx", bufs=1) as tailx,
            tc.tile_pool(name="opool", bufs=3) as opool,
            tc.tile_pool(name="psum", bufs=6, space="PSUM") as psum,
        ):
            # first x bytes in flight before anything else
            xa0 = xpool.tile([P, KO, CH], f8, tag="x0a", bufs=1)
            nc.sync.dma_start(xa0[:], xt0a_d[:])
            xb0 = xpool.tile([P, KO, 3 * CH], f8, tag="x0b", bufs=1)
            nc.sync.dma_start(xb0[:], xt0b_d[:])

            # consts on the ACT ring (contiguous layout: cheap descriptors)
            wt = consts.tile([P, KO, 2 * C], f8)
            nc.scalar.dma_start(wt[:], wt_d[:])

            # Engine warmups: give PE and ACT one instruction that observes
            # the const DMA lane so steady-state instructions carry at most
            # one sync wait each.
            scratch = psum.tile([P, CH], f32, tag="y")
            nc.tensor.matmul(
                scratch[:2, :2], wt[:, 0, :2], wt[:, 0, :2], start=True, stop=True
            )
            act_scr = consts.tile([1, 2 * C], bf16)
            nc.scalar.copy(act_scr[:], wt[:1, 0, :])

            def quad_mms(bank, rhs_of_chunk, n_chunks, ncols, k_lo=0, k_hi=KO,
                         first=True, last=True):
                # hi and lo accumulate into the SAME output partitions:
                # PSUM does the merge. 4-way col tiling at positions 32s.
                for ko in range(k_lo, k_hi):
                    for hl in range(2):
                        w = wt[:, ko, C * hl : C * (hl + 1)]
                        for s in range(n_chunks):
                            nc.tensor.matmul(
                                bank[32 * s : 32 * s + C, :ncols],
                                w,
                                rhs_of_chunk(ko, s),
                                start=(first and ko == k_lo and hl == 0 and True),
                                stop=(last and ko == k_hi - 1 and hl == 1 and True),
                                tile_position=(0, 32 * s),
                                skip_group_check=True,
                            )

            for m in range(NQ - 1):
                g, q = m // GQ, m % GQ
                if m == 0:
                    xq = None  # chunk 0 from xa0, chunks 1..3 from xb0
                else:
                    xq = xpool.tile([P, KO, QR], f8, tag="xq")
                    # all x on the SP ring: the ACT sequencer owns the out
                    # DMAs, and x triggers must never queue behind them
                    nc.sync.dma_start(xq[:], xt_d[m - 1])
                if q == 0:
                    ncols = GQ * CH if g < 3 else 3 * CH
                    out_g = opool.tile([P, ncols], f8, tag="out")
                bank = psum.tile([P, CH], f32, tag="y")
                if m == 0:
                    rhs = lambda ko, s: (
                        xa0[:, ko, :] if s == 0 else xb0[:, ko, CH * (s - 1) : CH * s]
                    )
                else:
                    rhs = lambda ko, s, _x=xq: _x[:, ko, CH * s : CH * (s + 1)]
                quad_mms(bank, rhs, 4, CH)
                # one scaled cast merges nothing — PSUM already holds hi+lo;
                # it just divides by 128 and narrows to e3m4
                nc.scalar.mul(out_g[:, CH * q : CH * (q + 1)], bank[:], INV_WSCALE)
                if g < 3 and q == GQ - 1:
                    nc.scalar.dma_start(out_d[g], out_g[:])
                elif g == 3 and q == 2:
                    nc.scalar.dma_start(outm_d[:], out_g[:])

            # ---- final quad: 1024 + 512 + 512(k-split) rows ----
            xty = tailx.tile([P, KO, 2 * CH], f8)
            nc.sync.dma_start(xty[:], xty_d[:])
            xz0 = tailx.tile([P, KO, CH], f8)
            nc.sync.dma_start(xz0[:], xtz0_d[:])
            xza = tailx.tile([P, KO // 2, CH], f8)
            nc.sync.dma_start(xza[:], xtzk_d[0])
            xzb = tailx.tile([P, KO // 2, CH], f8)
            nc.sync.dma_start(xzb[:], xtzk_d[1])

            # xty: 2 chunks of 512 rows at positions 0/32
            ybank = psum.tile([P, CH], f32, tag="y")
            quad_mms(ybank, lambda ko, s: xty[:, ko, CH * s : CH * (s + 1)], 2, CH)
            oy = opool.tile([2 * C, CH], f8, tag="oy")
            nc.scalar.mul(oy[:], ybank[: 2 * C, :], INV_WSCALE)
            nc.sync.dma_start(outy_d[:], oy[:])

            # xtz0: 4 sub-chunks of 128 rows at positions 0/32/64/96
            zbank = psum.tile([P, CH], f32, tag="y")
            quad_mms(
                zbank,
                lambda ko, s: xz0[:, ko, (CH // 4) * s : (CH // 4) * (s + 1)],
                4,
                CH // 4,
            )
            oz0 = opool.tile([P, CH // 4], f8, tag="oz")
            nc.scalar.mul(oz0[:], zbank[:, : CH // 4], INV_WSCALE)
            nc.sync.dma_start(outz0_d[:], oz0[:])

            # final 512 rows, k-tiles 0..3 from xza run while xzb streams;
            # only k-tiles 4..7 (N=128 x 4-way) trail the last x byte
            kbank = psum.tile([P, CH], f32, tag="y")
            quad_mms(
                kbank,
                lambda ko, s: xza[:, ko, (CH // 4) * s : (CH // 4) * (s + 1)],
                4,
                CH // 4,
                k_lo=0,
                k_hi=KO // 2,
                last=False,
            )
            quad_mms(
                kbank,
                lambda ko, s: xzb[:, ko - KO // 2, (CH // 4) * s : (CH // 4) * (s + 1)],
                4,
                CH // 4,
                k_lo=KO // 2,
                k_hi=KO,
                first=False,
            )
            ozk = opool.tile([P, CH // 4], f8, tag="oz")
            nc.scalar.mul(ozk[:], kbank[:, : CH // 4], INV_WSCALE)
            nc.sync.dma_start(outzk_d[:], ozk[:])
    nc.compile()
    return nc


_NC = None


def _get_nc():
    global _NC
    if _NC is None:
        _NC = _build()
    return _NC


def kernel(x, task_labels, W, b):
    global LAST_RESULTS, LAST_IN_MAPS
    x = np.asarray(x)
    if x.dtype != np.float32:
        x = x.astype(np.float32)
    labels = np.asarray(task_labels).astype(np.int64)
    W = np.asarray(W)
    if W.dtype != np.float32:
        W = W.astype(np.float32)
    b = np.asarray(b)
    if b.dtype != np.float32:
        b = b.astype(np.float32)

    order = np.argsort(labels, kind="stable")  # rows grouped by task
    counts = np.bincount(labels, minlength=T)
    starts = np.concatenate([[0], np.cumsum(counts)])

    in_maps = []
    over_rows = []  # (task, global row indices beyond capacity)
    for t in range(T):
        seg_idx = order[starts[t] : starts[t + 1]]
        n_dev = min(counts[t], NCAP)
        xs = np.zeros((NCAP, D), dtype=F8)
        xs[:n_dev] = x[seg_idx[:n_dev]]
        # xt[m, ki, ko, r] = xs[m*QR + r, ko*P + ki]
        xsq = xs.reshape(NQ, QR, KO, P)
        xt0 = np.ascontiguousarray(xsq[0].transpose(2, 1, 0))  # [P, KO, QR]
        xt0a = np.ascontiguousarray(xt0[:, :, :CH])
        xt0b = np.ascontiguousarray(xt0[:, :, CH:])
        xt = np.ascontiguousarray(xsq[1 : NQ - 1].transpose(0, 3, 2, 1))
        xty = np.ascontiguousarray(
            xs[(NQ - 1) * QR : NCAP - 2 * CH].reshape(2 * CH, KO, P).transpose(2, 1, 0)
        )
        xtz0 = np.ascontiguousarray(
            xs[NCAP - 2 * CH : NCAP - CH].reshape(CH, KO, P).transpose(2, 1, 0)
        )
        ztail = xs[NCAP - CH :].reshape(CH, KO, P)  # [r, ko, ki]
        xtzk = np.ascontiguousarray(
            ztail.reshape(CH, 2, KO // 2, P).transpose(1, 3, 2, 0)
        )
        ws = W[t].astype(np.float64) * WSCALE
        hi = np.clip(ws, -F8LIM, F8LIM).astype(F8)
        lo = np.clip(ws - hi.astype(np.float64), -F8LIM, F8LIM).astype(F8)
        # wt[ki, ko, 0:32]=hi[c, ko*128+ki], [ki, ko, 32:64]=lo
        wt = np.empty((P, KO, 2 * C), dtype=F8)
        wt[:, :, :C] = hi.T.reshape(KO, P, C).transpose(1, 0, 2)
        wt[:, :, C:] = lo.T.reshape(KO, P, C).transpose(1, 0, 2)
        in_maps.append(
            {
                "xt0a": xt0a,
                "xt0b": xt0b,
                "xt": xt,
                "xty": xty,
                "xtz0": xtz0,
                "xtzk": xtzk,
                "wt": np.ascontiguousarray(wt),
            }
        )
        if counts[t] > NCAP:
            over_rows.append((t, seg_idx[NCAP:]))

    LAST_IN_MAPS = in_maps
    nc = _get_nc()
    res = bass_utils.run_bass_kernel_spmd(
        nc, in_maps, core_ids=list(range(NCORES)), trace=TRACE
    )
    LAST_RESULTS = res

    out = np.empty((B, C), dtype=np.float32)
    for t in range(T):
        seg_idx = order[starts[t] : starts[t + 1]]
        n_dev = min(counts[t], NCAP)
        o = np.empty((NCAP, C), dtype=np.float32)
        r = res.results[t]
        # out_d[g, 32j+c, 512q+r] -> row 2048*(4g+q) + 512j + r
        og = np.asarray(r["out"]).astype(np.float32)
        o[: 12 * QR] = (
            og.reshape(3, 4, C, GQ, CH).transpose(0, 3, 1, 4, 2).reshape(12 * QR, C)
        )
        om = np.asarray(r["outm"]).astype(np.float32)
        o[12 * QR : 15 * QR] = (
            om.reshape(4, C, 3, CH).transpose(2, 0, 3, 1).reshape(3 * QR, C)
        )
        oy = np.asarray(r["outy"]).astype(np.float32)
        o[15 * QR : 15 * QR + 2 * CH] = (
            oy.reshape(2, C, CH).transpose(0, 2, 1).reshape(2 * CH, C)
        )
        oz0 = np.asarray(r["outz0"]).astype(np.float32)
        o[NCAP - 2 * CH : NCAP - CH] = (
            oz0.reshape(4, C, CH // 4).transpose(0, 2, 1).reshape(CH, C)
        )
        ozk = np.asarray(r["outzk"]).astype(np.float32)
        o[NCAP - CH :] = (
            ozk.reshape(4, C, CH // 4).transpose(0, 2, 1).reshape(CH, C)
        )
        out[seg_idx[:n_dev]] = o[:n_dev]
    for t, idx in over_rows:
        out[idx] = x[idx] @ W[t].T
    out += b[labels]
    return out


# revision 5
# speedup vs baseline: 1.1266x; 1.1266x over previous
"""Trainium2 kernel for nn_MultiHeadClassifier.

Math: out[i] = W[task_labels[i]] @ x[i] + b[task_labels[i]]
  x [262144, 1024] f32, task_labels [262144] int, W [8, 32, 1024], b [8, 32]

Strategy (8 NeuronCores, task-parallel, fp8 x, fp8 out):
  - Host sorts rows by task; core c processes (up to NCAP=32768) rows of
    task c, so W[c] is a per-core constant and there is NO routing on
    device at all — each core runs a plain GEMM. The ~few hundred rows
    that overflow a core's capacity are computed on host (numpy) and the
    result is merged back; bias is added on host.
  - x is sent as fp8 e3m4 (1 byte: 4 mantissa bits), quartering the
    dominant HBM traffic vs f32. W is scaled by 128 and sent as an
    e3m4 hi+lo pair, which cancels the W quantization error to second
    order. hi and lo live side by side in one M=64 stationary
    [128, 64], so each k-tile needs ONE matmul; two 512-row chunks
    pack into each PSUM bank via column tiling (positions 0/64), which
    the PE streams 2-way concurrently (the XBUS ceiling: 4-way M=32
    was HW-measured at only ~2.3x effective and is net slower).
  - The per-core stream is SDMA-engine-bound (~26.5 GB/s x 16 engines
    ~= 420 GB/s), so output bytes matter: per quad, ACT copies hi to a
    bf16 staging tile (473 ns/chunk), DVE adds lo (424 ns/chunk), and
    ACT casts the [128, 512] block * (1/128) to e3m4 (~0.7 us), so
    quads 0..14 ship as fp8 — 0.94 MiB vs 2.75 MiB for the bf16+raw
    scheme. Engine load stays < 60% of the 4.7 us/quad cadence.
    Measured rel err ~1.79e-2 vs the 2e-2 gate (HW-validated sim).
  - x streams on the SP HWDGE ring; the first transfer is 0.5 MiB so
    data flows ~1 us earlier. fp8 out groups ship on the ACT ring
    (2 KiB/partition lines). The last 2048 rows ship their PSUM banks
    raw (bf16, host adds hi+lo; +0.19 MiB) so no merge chain sits in
    the tail; the final 512 rows arrive split by k-tile halves so only
    4 k-tiles of N=256 matmuls + one copy + one 64 KiB DMA trail the
    last x byte.
"""

import sys

sys.path.insert(0, "/opt/trn_rl_repo")

import numpy as np
import ml_dtypes

import concourse.bass as bass
import concourse.tile as tile
from concourse import bacc, mybir
from concourse import bass_utils

B, D, C, T = 262144, 1024, 32, 8
NCORES = 8
P = 128
KO = D // P  # 8 contraction k-tiles
CH = 512  # rows per chunk (one PSUM column-tile)
QR = 2048  # rows per quad (one x DMA; 4 chunks in 2 PSUM banks)
NQ = 16  # quads per core
NCAP = QR * NQ  # 32768 rows per core capacity
GQ = 4  # quads per output DMA group
WSCALE = 128.0  # power of two; exactly cancelled by the on-device 1/128
INV_WSCALE = 1.0 / WSCALE

F8 = ml_dtypes.float8_e3m4
F8LIM = 15.5  # max finite e3m4

# set by test harness to collect a profile; harness-invoked kernel() keeps it off
TRACE = False
LAST_RESULTS = None
LAST_IN_MAPS = None


def _build():
    f32 = mybir.dt.float32
    bf16 = mybir.dt.bfloat16
    f8 = mybir.dt.float8e3

    nc = bacc.Bacc("TRN2", debug=False, num_devices=NCORES)
    # quad 0 split 512+1536 rows: small first transfer -> first byte ~1us
    # earlier (descriptor gen + HBM latency off the critical path)
    xt0a_d = nc.dram_tensor("xt0a", [P, KO, CH], f8, kind="ExternalInput")
    xt0b_d = nc.dram_tensor("xt0b", [P, KO, 3 * CH], f8, kind="ExternalInput")
    # body quads 1..14: contiguous 2 MB, 16 KB/partition
    xt_d = nc.dram_tensor("xt", [NQ - 2, P, KO, QR], f8, kind="ExternalInput")
    # final quad: 1024 rows, then 512, then 512 split by k-halves
    xty_d = nc.dram_tensor("xty", [P, KO, 2 * CH], f8, kind="ExternalInput")
    xtz0_d = nc.dram_tensor("xtz0", [P, KO, CH], f8, kind="ExternalInput")
    xtzk_d = nc.dram_tensor("xtzk", [2, P, KO // 2, CH], f8, kind="ExternalInput")
    # wt[ki, ko, 0:32]=hi, [ki, ko, 32:64]=lo (host-transposed, scaled)
    wt_d = nc.dram_tensor("wt", [P, KO, 2 * C], f8, kind="ExternalInput")
    # merged fp8 output (already divided by 128 on device):
    #   out_d[g, 32j+c, 512q+r] -> row 2048*(4g+q) + 512j + r   (quads 0..11)
    out_d = nc.dram_tensor("out", [3, P, GQ * CH], f8, kind="ExternalOutput")
    #   outm_d[32j+c, 512q+r] -> row 2048*(12+q) + 512j + r     (quads 12..14)
    outm_d = nc.dram_tensor("outm", [P, 3 * CH], f8, kind="ExternalOutput")
    # last 2048 rows raw bf16 (host adds hi+lo, divides by 128):
    #   outy_d[64jj+{c|32+c}, r] -> row 30720 + 512jj + r       (jj=0,1)
    outy_d = nc.dram_tensor("outy", [P, CH], bf16, kind="ExternalOutput")
    #   outz0_d[64h+{c|32+c}, r] -> row 31744 + 256h + r        (h=0,1)
    outz0_d = nc.dram_tensor("outz0", [P, CH // 2], bf16, kind="ExternalOutput")
    #   outzk_d[64h+{c|32+c}, r] -> row 32256 + 256h + r
    outzk_d = nc.dram_tensor("outzk", [P, CH // 2], bf16, kind="ExternalOutput")

    with tile.TileContext(nc) as tc:
        with (
            tc.tile_pool(name="consts", bufs=1) as consts,
            tc.tile_pool(name="xpool", bufs=8) as xpool,
            tc.tile_pool(name="tailx", bufs=1) as tailx,
            tc.tile_pool(name="stage", bufs=4) as stage,
            tc.tile_pool(name="opool", bufs=3) as opool,
            tc.tile_pool(name="psum", bufs=6, space="PSUM") as psum,
            tc.tile_pool(name="psumz", bufs=2, space="PSUM") as psumz,
        ):
            # first x bytes in flight before anything else
            xa0 = xpool.tile([P, KO, CH], f8, tag="x0a", bufs=1)
            nc.sync.dma_start(xa0[:], xt0a_d[:])
            xb0 = xpool.tile([P, KO, 3 * CH], f8, tag="x0b", bufs=1)
            nc.sync.dma_start(xb0[:], xt0b_d[:])

            # consts on the ACT ring (contiguous layout: cheap descriptors)
            wt = consts.tile([P, KO, 2 * C], f8)
            nc.scalar.dma_start(wt[:], wt_d[:])

            # Engine warmups: give PE and DVE one instruction that observes
            # the const DMA lane so steady-state instructions carry at most
            # one sync wait each.
            scratch = psum.tile([P, CH], f32, tag="y4")
            nc.tensor.matmul(
                scratch[:2, :2], wt[:, 0, :2], wt[:, 0, :2], start=True, stop=True
            )
            dve_scr = consts.tile([1, 2 * C], bf16)
            nc.vector.tensor_copy(dve_scr[:], wt[:1, 0, :])

            for m in range(NQ - 1):
                g, q = m // GQ, m % GQ
                if m == 0:
                    xq = None  # chunk 0 from xa0, chunks 1..3 from xb0
                else:
                    xq = xpool.tile([P, KO, QR], f8, tag="xq")
                    # all x on the SP ring: the ACT sequencer owns the hi
                    # copies + out DMAs; x triggers must never queue there
                    nc.sync.dma_start(xq[:], xt_d[m - 1])
                if q == 0:
                    ncols = GQ * CH if g < 3 else 3 * CH
                    out_g = opool.tile([P, ncols], f8, tag="out")
                # 2 banks x 2 column positions = 4 chunks of 512 rows
                ya = psum.tile([P, CH], f32, tag="y4")
                yb = psum.tile([P, CH], f32, tag="y4")
                for ko in range(KO):
                    for bank, (j0, j1) in ((ya, (0, 1)), (yb, (2, 3))):
                        for pos, j in ((0, j0), (64, j1)):
                            if m == 0:
                                rhs = (
                                    xa0[:, ko, :]
                                    if j == 0
                                    else xb0[:, ko, CH * (j - 1) : CH * j]
                                )
                            else:
                                rhs = xq[:, ko, CH * j : CH * (j + 1)]
                            nc.tensor.matmul(
                                bank[pos : pos + 2 * C, :],
                                wt[:, ko, :],
                                rhs,
                                start=(ko == 0),
                                stop=(ko == KO - 1),
                                tile_position=(0, pos),
                                skip_group_check=True,
                            )
                # out = (hi + lo)/128 as e3m4: ACT copies hi into bf16
                # staging, DVE adds lo (one PSUM operand per instruction),
                # ACT casts the whole [128, 512] block with scale
                st = stage.tile([P, CH], bf16, tag="st")
                for j in range(4):
                    bank = ya if j < 2 else yb
                    base = 64 * (j % 2)
                    sl = st[C * j : C * (j + 1), :]
                    nc.scalar.copy(sl, bank[base : base + C, :])
                    nc.vector.tensor_tensor(
                        sl, sl, bank[base + C : base + 2 * C, :], mybir.AluOpType.add
                    )
                nc.scalar.mul(out_g[:, CH * q : CH * (q + 1)], st[:], INV_WSCALE)
                if g < 3 and q == GQ - 1:
                    nc.scalar.dma_start(out_d[g], out_g[:])
                elif g == 3 and q == 2:
                    nc.scalar.dma_start(outm_d[:], out_g[:])

            # ---- final 2048 rows: raw bf16 banks, no merge in the tail ----
            xty = tailx.tile([P, KO, 2 * CH], f8)
            nc.sync.dma_start(xty[:], xty_d[:])
            xz0 = tailx.tile([P, KO, CH], f8)
            nc.sync.dma_start(xz0[:], xtz0_d[:])
            xza = tailx.tile([P, KO // 2, CH], f8)
            nc.sync.dma_start(xza[:], xtzk_d[0])
            xzb = tailx.tile([P, KO // 2, CH], f8)
            nc.sync.dma_start(xzb[:], xtzk_d[1])

            # xty: 2 chunks of 512 rows (positions 0/64), one bank, raw ship
            ya = psum.tile([P, CH], f32, tag="y4")
            for ko in range(KO):
                for pos, cix in ((0, 0), (64, 1)):
                    nc.tensor.matmul(
                        ya[pos : pos + 2 * C, :],
                        wt[:, ko, :],
                        xty[:, ko, CH * cix : CH * (cix + 1)],
                        start=(ko == 0),
                        stop=(ko == KO - 1),
                        tile_position=(0, pos),
                        skip_group_check=True,
                    )
            zy = opool.tile([P, CH], bf16, tag="piece")
            nc.scalar.copy(zy[:], ya[:])
            nc.sync.dma_start(outy_d[:], zy[:])

            # xz0: one 512-row piece as a 2x256 pair (N=256), raw ship
            yb = psumz.tile([P, CH // 2], f32, tag="yz")
            for ko in range(KO):
                for pos in (0, 64):
                    nc.tensor.matmul(
                        yb[pos : pos + 2 * C, :],
                        wt[:, ko, :],
                        xz0[:, ko, CH // 2 * (pos // 64) : CH // 2 * (pos // 64) + CH // 2],
                        start=(ko == 0),
                        stop=(ko == KO - 1),
                        tile_position=(0, pos),
                        skip_group_check=True,
                    )
            zb = opool.tile([P, CH // 2], bf16, tag="piece")
            nc.scalar.copy(zb[:], yb[:])
            nc.sync.dma_start(outz0_d[:], zb[:])

            # final 512 rows: k-tiles 0..3 compute while the second half
            # streams; only k-tiles 4..7 + one DVE copy + one 64 KiB DMA
            # trail the last x byte
            yc = psumz.tile([P, CH // 2], f32, tag="yz")
            for srcx, klo in ((xza, 0), (xzb, KO // 2)):
                for ko in range(klo, klo + KO // 2):
                    for pos in (0, 64):
                        nc.tensor.matmul(
                            yc[pos : pos + 2 * C, :],
                            wt[:, ko, :],
                            srcx[
                                :,
                                ko - klo,
                                CH // 2 * (pos // 64) : CH // 2 * (pos // 64) + CH // 2,
                            ],
                            start=(ko == 0),
                            stop=(ko == KO - 1),
                            tile_position=(0, pos),
                            skip_group_check=True,
                        )
            zc = opool.tile([P, CH // 2], bf16, tag="piece")
            nc.vector.tensor_copy(zc[:], yc[:])
            nc.sync.dma_start(outzk_d[:], zc[:])
    nc.compile()
    return nc


_NC = None


def _get_nc():
    global _NC
    if _NC is None:
        _NC = _build()
    return _NC


def kernel(x, task_labels, W, b):
    global LAST_RESULTS, LAST_IN_MAPS
    x = np.asarray(x)
    if x.dtype != np.float32:
        x = x.astype(np.float32)
    labels = np.asarray(task_labels).astype(np.int64)
    W = np.asarray(W)
    if W.dtype != np.float32:
        W = W.astype(np.float32)
    b = np.asarray(b)
    if b.dtype != np.float32:
        b = b.astype(np.float32)

    order = np.argsort(labels, kind="stable")  # rows grouped by task
    counts = np.bincount(labels, minlength=T)
    starts = np.concatenate([[0], np.cumsum(counts)])

    in_maps = []
    over_rows = []  # (task, global row indices beyond capacity)
    for t in range(T):
        seg_idx = order[starts[t] : starts[t + 1]]
        n_dev = min(counts[t], NCAP)
        xs = np.zeros((NCAP, D), dtype=F8)
        xs[:n_dev] = x[seg_idx[:n_dev]]
        # xt[m, ki, ko, r] = xs[m*QR + r, ko*P + ki]
        xsq = xs.reshape(NQ, QR, KO, P)
        xt0 = np.ascontiguousarray(xsq[0].transpose(2, 1, 0))  # [P, KO, QR]
        xt0a = np.ascontiguousarray(xt0[:, :, :CH])
        xt0b = np.ascontiguousarray(xt0[:, :, CH:])
        xt = np.ascontiguousarray(xsq[1 : NQ - 1].transpose(0, 3, 2, 1))
        xty = np.ascontiguousarray(
            xs[(NQ - 1) * QR : NCAP - 2 * CH].reshape(2 * CH, KO, P).transpose(2, 1, 0)
        )
        xtz0 = np.ascontiguousarray(
            xs[NCAP - 2 * CH : NCAP - CH].reshape(CH, KO, P).transpose(2, 1, 0)
        )
        ztail = xs[NCAP - CH :].reshape(CH, KO, P)  # [r, ko, ki]
        xtzk = np.ascontiguousarray(
            ztail.reshape(CH, 2, KO // 2, P).transpose(1, 3, 2, 0)
        )
        ws = W[t].astype(np.float64) * WSCALE
        hi = np.clip(ws, -F8LIM, F8LIM).astype(F8)
        lo = np.clip(ws - hi.astype(np.float64), -F8LIM, F8LIM).astype(F8)
        # wt[ki, ko, 0:32]=hi[c, ko*128+ki], [ki, ko, 32:64]=lo
        wt = np.empty((P, KO, 2 * C), dtype=F8)
        wt[:, :, :C] = hi.T.reshape(KO, P, C).transpose(1, 0, 2)
        wt[:, :, C:] = lo.T.reshape(KO, P, C).transpose(1, 0, 2)
        in_maps.append(
            {
                "xt0a": xt0a,
                "xt0b": xt0b,
                "xt": xt,
                "xty": xty,
                "xtz0": xtz0,
                "xtzk": xtzk,
                "wt": np.ascontiguousarray(wt),
            }
        )
        if counts[t] > NCAP:
            over_rows.append((t, seg_idx[NCAP:]))

    LAST_IN_MAPS = in_maps
    nc = _get_nc()
    res = bass_utils.run_bass_kernel_spmd(
        nc, in_maps, core_ids=list(range(NCORES)), trace=TRACE
    )
    LAST_RESULTS = res

    inv = np.float32(INV_WSCALE)
    out = np.empty((B, C), dtype=np.float32)
    for t in range(T):
        seg_idx = order[starts[t] : starts[t + 1]]
        n_dev = min(counts[t], NCAP)
        o = np.empty((NCAP, C), dtype=np.float32)
        r = res.results[t]
        # out_d[g, 32j+c, 512q+r] -> row 2048*(4g+q) + 512j + r
        og = np.asarray(r["out"]).astype(np.float32)
        o[: 12 * QR] = (
            og.reshape(3, 4, C, GQ, CH).transpose(0, 3, 1, 4, 2).reshape(12 * QR, C)
        )
        om = np.asarray(r["outm"]).astype(np.float32)
        o[12 * QR : 15 * QR] = (
            om.reshape(4, C, 3, CH).transpose(2, 0, 3, 1).reshape(3 * QR, C)
        )
        # raw pieces: [64jj+{c|32+c}, r]; value = (hi + lo)/128
        oy = np.asarray(r["outy"]).astype(np.float32).reshape(2, 2, C, CH)
        o[15 * QR : 15 * QR + 2 * CH] = (
            ((oy[:, 0] + oy[:, 1]) * inv).transpose(0, 2, 1).reshape(2 * CH, C)
        )
        oz0 = np.asarray(r["outz0"]).astype(np.float32).reshape(2, 2, C, CH // 2)
        o[NCAP - 2 * CH : NCAP - CH] = (
            ((oz0[:, 0] + oz0[:, 1]) * inv).transpose(0, 2, 1).reshape(CH, C)
        )
        ozk = np.asarray(r["outzk"]).astype(np.float32).reshape(2, 2, C, CH // 2)
        o[NCAP - CH :] = (
            ((ozk[:, 0] + ozk[:, 1]) * inv).transpose(0, 2, 1).reshape(CH, C)
        )
        out[seg_idx[:n_dev]] = o[:n_dev]
    for t, idx in over_rows:
        out[idx] = x[idx] @ W[t].T
    out += b[labels]
    return out
